# revision 1
# baseline (speedup 1.0000x reference)
"""TRN2 Bass/Tile kernel for nn_EngramUpsampler (dense_mlp).

Reference computation:
    x   = repeat_interleave(engrams, 32, axis=2) + pos_emb   # (B, NW, 512, 1024)
    h   = gelu(x @ w1.T + b1, exact)
    x   = x + h @ w2.T + b2
    out = LayerNorm(x) * gamma + beta

Distribution: data-parallel over the B*NW = 64 (batch, window) slices,
8 windows per NeuronCore; weights/pos_emb replicated.

Per-core kernel design (bf16 matmuls, fp32 PSUM accumulate + fp32 LN):
  * The first matmul factors through the repeat:
        x @ w1.T = repeat(eng @ w1.T) + pos @ w1.T
    eng_proj (128 tokens/core) and pos_projT (512 shared tokens) are computed
    once per core -- a ~21x FLOP reduction on matmul 1.  The repeat+add is done
    on the TensorEngine by accumulating  R.T @ eng_proj + I.T @ pos_projT  into
    PSUM, where R is the 16x512 0/1 repeat matrix, keeping DVE free.
  * h = gelu(psum) on ScalarE (erf-exact Gelu LUT), emitted as bf16.
  * Second matmul h @ w2.T accumulates per 128-token tile into PSUM; the
    residual x0 = repeat(eng) + pos joins the same accumulation via R/I
    matmuls (and b2 via a ones-row matmul when nonzero).
  * LayerNorm: bn_stats/bn_aggr on DVE; rsqrt via Newton iteration on DVE
    (avoids ACT table-set switches between Gelu and Sqrt); normalization on
    GpSimd (one-input streaming op, runs at line rate); gamma/beta passes are
    emitted only when they are non-trivial (reference uses gamma=1, beta=0).
"""

import numpy as np
import ml_dtypes

import concourse.bass as bass
import concourse.tile as tile
from concourse import bacc, mybir
from concourse.bass_utils import run_bass_kernel_spmd

FP32 = mybir.dt.float32
BF16 = mybir.dt.bfloat16
AF = mybir.ActivationFunctionType
OP = mybir.AluOpType

B, NW, K, D, W = 4, 16, 16, 1024, 512
REP = W // K          # 32
N_CORES = 8
WPC = (B * NW) // N_CORES   # 8 windows per core
TOK = WPC * W               # 4096 tokens per core
DC = D // 128               # 8 chunks of the contraction dim
LN_EPS = 1e-5
NEWTON_ITERS = 5

_PROGRAM_CACHE: dict = {}


def _build_program(has_b1: bool, has_b2: bool, has_gb: bool):
    """Emit + compile the per-core SPMD program. Same program runs on all 8
    cores; only the DRAM input contents differ."""
    from contextlib import ExitStack

    nc = bacc.Bacc("TRN2", target_bir_lowering=False, debug=False)

    eng_d = nc.dram_tensor("eng", [WPC * K, D], BF16, kind="ExternalInput").ap()
    pos_d = nc.dram_tensor("pos", [W, D], BF16, kind="ExternalInput").ap()
    w1t_d = nc.dram_tensor("w1t", [D, D], BF16, kind="ExternalInput").ap()
    w2t_d = nc.dram_tensor("w2t", [D, D], BF16, kind="ExternalInput").ap()
    rep_d = nc.dram_tensor("rep", [128, W], BF16, kind="ExternalInput").ap()
    eng8_d = nc.dram_tensor("eng8", [WPC * 128, D], BF16, kind="ExternalInput").ap()
    idn_d = nc.dram_tensor("idn", [128, 128], BF16, kind="ExternalInput").ap()
    if has_b1:
        b1c_d = nc.dram_tensor("b1c", [128, DC], FP32, kind="ExternalInput").ap()
    if has_b2:
        b2r_d = nc.dram_tensor("b2r", [1, D], BF16, kind="ExternalInput").ap()
    if has_gb:
        gam_d = nc.dram_tensor("gam", [1, D], FP32, kind="ExternalInput").ap()
        bet_d = nc.dram_tensor("bet", [1, D], FP32, kind="ExternalInput").ap()
    out_d = nc.dram_tensor("out", [TOK, D], FP32, kind="ExternalOutput").ap()

    with tile.TileContext(nc) as tc, ExitStack() as ctx:
        consts = ctx.enter_context(tc.tile_pool(name="consts", bufs=1))

        def ctile(shape, dtype, tag):
            return consts.tile(shape, dtype, tag=tag, name=tag)

        # ---- persistent SBUF tensors -------------------------------------
        w1t_sb = [ctile([128, D], BF16, f"w1t{i}") for i in range(DC)]
        w2t_sb = [ctile([128, D], BF16, f"w2t{i}") for i in range(DC)]
        pos_sb = [ctile([128, D], BF16, f"pos{i}") for i in range(4)]
        eng_all = ctile([128, D], BF16, "eng_all")
        eng8_sb = [ctile([128, D], BF16, f"engw{i}") for i in range(WPC)]
        r_sb = ctile([128, W], BF16, "r_sb")
        id_sb = ctile([128, 128], BF16, "id_sb")
        engT = ctile([128, D], BF16, "engT")            # [d, t] chunks in cols
        posT = [ctile([128, W], BF16, f"posT{i}") for i in range(DC)]
        engproj_all = ctile([128, D], BF16, "engproj")  # [t, e]
        engprojT = ctile([128, D], BF16, "engprojT")    # [e_local, token] per ec block
        posproj = [ctile([128, W], BF16, f"ppj{i}") for i in range(DC)]  # [e, w]
        if has_b1:
            b1c_sb = ctile([128, DC], FP32, "b1c")
        if has_b2:
            ones_sb = ctile([1, 128], BF16, "ones")
            b2_sb = ctile([1, D], BF16, "b2")
        if has_gb:
            gam_row = ctile([1, D], FP32, "gamr")
            bet_row = ctile([1, D], FP32, "betr")
            gam_sb = ctile([128, D], FP32, "gam")
            bet_sb = ctile([128, D], FP32, "bet")

        # ---- loads -------------------------------------------------------
        nc.sync.dma_start(id_sb[:], idn_d[:])
        nc.sync.dma_start(r_sb[:], rep_d[:])
        nc.sync.dma_start(eng_all[:], eng_d[:])
        for i in range(4):
            nc.sync.dma_start(pos_sb[i][:], pos_d[i * 128:(i + 1) * 128, :])
        for i in range(DC):
            nc.sync.dma_start(w1t_sb[i][:], w1t_d[i * 128:(i + 1) * 128, :])
        for n in range(WPC):
            nc.sync.dma_start(eng8_sb[n][:], eng8_d[n * 128:(n + 1) * 128, :])
        for i in range(DC):
            nc.sync.dma_start(w2t_sb[i][:], w2t_d[i * 128:(i + 1) * 128, :])
        if has_b1:
            nc.sync.dma_start(b1c_sb[:], b1c_d[:])
        if has_b2:
            nc.gpsimd.memset(ones_sb[:], 1.0)
            nc.sync.dma_start(b2_sb[:], b2r_d[:])
        if has_gb:
            nc.sync.dma_start(gam_row[:], gam_d[:])
            nc.sync.dma_start(bet_row[:], bet_d[:])
            nc.gpsimd.partition_broadcast(gam_sb[:], gam_row[:])
            nc.gpsimd.partition_broadcast(bet_sb[:], bet_row[:])

        # ---- setup: transposes + factored first matmul -------------------
        with tc.tile_pool(name="spsum", bufs=2, space="PSUM") as spsum:
            # engT[d, t]: PE-transpose eng 128x128 blocks
            for dc in range(DC):
                pt = spsum.tile([128, 128], BF16, tag="pt", name="pt")
                nc.tensor.transpose(pt[:], eng_all[:, dc * 128:(dc + 1) * 128], id_sb[:])
                nc.vector.tensor_copy(engT[:, dc * 128:(dc + 1) * 128], pt[:])
            # posT[dc] = [d, w]: transpose pos tiles
            for g in range(4):
                for dc in range(DC):
                    pt = spsum.tile([128, 128], BF16, tag="pt", name="pt")
                    nc.tensor.transpose(
                        pt[:], pos_sb[g][:, dc * 128:(dc + 1) * 128], id_sb[:]
                    )
                    nc.vector.tensor_copy(posT[dc][:, g * 128:(g + 1) * 128], pt[:])
            # eng_proj[t, e] = engT.T @ w1t  (accumulate over d chunks)
            ep = spsum.tile([128, D], FP32, tag="ep", name="ep")
            for half in range(2):
                sl = slice(half * 512, half * 512 + 512)
                for dc in range(DC):
                    nc.tensor.matmul(
                        ep[:, sl],
                        lhsT=engT[:, dc * 128:(dc + 1) * 128],
                        rhs=w1t_sb[dc][:, sl],
                        start=(dc == 0),
                        stop=(dc == DC - 1),
                    )
            nc.vector.tensor_copy(engproj_all[:], ep[:])
            for ec in range(DC):
                pt = spsum.tile([128, 128], BF16, tag="pt", name="pt")
                nc.tensor.transpose(
                    pt[:], engproj_all[:, ec * 128:(ec + 1) * 128], id_sb[:]
                )
                nc.vector.tensor_copy(engprojT[:, ec * 128:(ec + 1) * 128], pt[:])
            # pos_projT[e, w] per e-chunk = w1t_chunk.T @ posT
            for ec in range(DC):
                pp = spsum.tile([128, W], FP32, tag="pp", name="pp")
                for dc in range(DC):
                    nc.tensor.matmul(
                        pp[:],
                        lhsT=w1t_sb[dc][:, ec * 128:(ec + 1) * 128],
                        rhs=posT[dc][:],
                        start=(dc == 0),
                        stop=(dc == DC - 1),
                    )
                nc.vector.tensor_copy(posproj[ec][:], pp[:])

        # ---- main loop ---------------------------------------------------
        pre_pool = ctx.enter_context(tc.tile_pool(name="pre", bufs=3))
        x_pool = ctx.enter_context(tc.tile_pool(name="xps", bufs=3, space="PSUM"))
        h_pool = ctx.enter_context(tc.tile_pool(name="h", bufs=2))
        xs_pool = ctx.enter_context(tc.tile_pool(name="xs", bufs=6))
        st_pool = ctx.enter_context(tc.tile_pool(name="st", bufs=4))
        nt_pool = ctx.enter_context(tc.tile_pool(name="nt", bufs=2))
        out_pool = ctx.enter_context(tc.tile_pool(name="ot", bufs=3))

        for n in range(WPC):
            # -- h = gelu(repeat(eng_proj) + pos_projT) per e-chunk --------
            h_tiles = []
            for ec in range(DC):
                pre = pre_pool.tile([128, W], FP32, tag="pre", name="pre")
                # pre1[e, w] = engprojT[e, k(w)] + posproj[e, w]; the repeat is
                # a stride-0 inner dim on the engprojT read.
                eng_bc = engprojT[:, ec * 128 + n * K: ec * 128 + (n + 1) * K]
                eng_bc = eng_bc.unsqueeze(2).broadcast_to([128, K, REP])
                nc.gpsimd.tensor_tensor(
                    pre[:].rearrange("p (k r) -> p k r", r=REP),
                    eng_bc,
                    posproj[ec][:].rearrange("p (k r) -> p k r", r=REP),
                    OP.add,
                )
                h = h_pool.tile([128, W], BF16, tag=f"h{ec}", name=f"h{ec}")
                bias = b1c_sb[:, ec:ec + 1] if has_b1 else 0.0
                nc.scalar.activation(h[:], pre[:], AF.Gelu, bias=bias)
                h_tiles.append(h)

            # -- second matmul + residual, then LN stats per token tile ----
            mvw = st_pool.tile([128, 2 * 4], FP32, tag="mvw", name="mvw")
            xs_tiles = []
            for g in range(4):
                px = x_pool.tile([128, D], FP32, tag="px", name="px")
                for half in range(2):
                    sl = slice(half * 512, half * 512 + 512)
                    for dc in range(DC):
                        nc.tensor.matmul(
                            px[:, sl],
                            lhsT=h_tiles[dc][:, g * 128:(g + 1) * 128],
                            rhs=w2t_sb[dc][:, sl],
                            start=(dc == 0),
                            stop=False,
                        )
                    nc.tensor.matmul(
                        px[:, sl],
                        lhsT=r_sb[:, g * 128:(g + 1) * 128],
                        rhs=eng8_sb[n][:, sl],
                        start=False,
                        stop=False,
                    )
                    if has_b2:
                        nc.tensor.matmul(
                            px[:, sl],
                            lhsT=ones_sb[:],
                            rhs=b2_sb[:, sl],
                            start=False,
                            stop=False,
                        )
                    nc.tensor.matmul(
                        px[:, sl],
                        lhsT=id_sb[:],
                        rhs=pos_sb[g][:, sl],
                        start=False,
                        stop=True,
                    )
                xs = xs_pool.tile([128, D], FP32, tag="xs", name="xs")
                nc.scalar.activation(xs[:], px[:], AF.Identity)
                st = st_pool.tile([128, 12], FP32, tag="st", name="st")
                nc.vector.bn_stats(st[:, 0:6], px[:, 0:512])
                nc.vector.bn_stats(st[:, 6:12], px[:, 512:1024])
                nc.vector.bn_aggr(
                    mvw[:, 2 * g:2 * g + 2],
                    st[:].rearrange("p (n s) -> p n s", s=3),
                )
                xs_tiles.append(xs)

            # -- rstd = rsqrt(var + eps), Newton on DVE, batched [128,4] ---
            vw = nt_pool.tile([128, 4], FP32, tag="vw", name="vw")
            nc.vector.tensor_scalar_add(vw[:], mvw[:, 1::2], LN_EPS)
            t0 = nt_pool.tile([128, 4], FP32, tag="t0", name="t0")
            nc.vector.tensor_scalar(t0[:], vw[:], 0.5, 0.5, OP.mult, OP.add)
            y = nt_pool.tile([128, 4], FP32, tag="y", name="y")
            nc.vector.reciprocal(y[:], t0[:])
            for _ in range(NEWTON_ITERS):
                y2 = nt_pool.tile([128, 4], FP32, tag="y2", name="y2")
                nc.vector.tensor_mul(y2[:], y[:], y[:])
                t = nt_pool.tile([128, 4], FP32, tag="t", name="t")
                nc.vector.tensor_mul(t[:], y2[:], vw[:])
                c = nt_pool.tile([128, 4], FP32, tag="c", name="c")
                nc.vector.tensor_scalar(c[:], t[:], -0.5, 1.5, OP.mult, OP.add)
                yn = nt_pool.tile([128, 4], FP32, tag="y", name="y")
                nc.vector.tensor_mul(yn[:], y[:], c[:])
                y = yn
            nm = nt_pool.tile([128, 4], FP32, tag="nm", name="nm")
            nc.vector.tensor_scalar_mul(nm[:], mvw[:, 0::2], -1.0)

            # -- normalize + store ----------------------------------------
            for g in range(4):
                row0 = n * W + g * 128
                if not has_gb:
                    ot = out_pool.tile([128, D], FP32, tag="ot", name="ot")
                    nc.gpsimd.tensor_scalar(
                        ot[:], xs_tiles[g][:],
                        nm[:, g:g + 1], y[:, g:g + 1], OP.add, OP.mult,
                    )
                else:
                    xn = out_pool.tile([128, D], FP32, tag="xn", name="xn")
                    nc.gpsimd.tensor_scalar(
                        xn[:], xs_tiles[g][:],
                        nm[:, g:g + 1], y[:, g:g + 1], OP.add, OP.mult,
                    )
                    ot = out_pool.tile([128, D], FP32, tag="ot", name="ot")
                    # ot = xn * gamma + beta
                    nc.vector.scalar_tensor_tensor(
                        ot[:], xn[:], 1.0, gam_sb[:], OP.mult, OP.mult
                    )
                    nc.vector.tensor_add(ot[:], ot[:], bet_sb[:])
                nc.sync.dma_start(out_d[row0:row0 + 128, :], ot[:])

    nc.compile()
    return nc


def _get_program(has_b1, has_b2, has_gb):
    key = (has_b1, has_b2, has_gb)
    if key not in _PROGRAM_CACHE:
        _PROGRAM_CACHE[key] = _build_program(*key)
    return _PROGRAM_CACHE[key]


def _make_in_maps(engrams, pos_emb, w1, b1, w2, b2, gamma, beta,
                  has_b1, has_b2, has_gb):
    bf16 = ml_dtypes.bfloat16
    eng_flat = np.asarray(engrams, np.float32).reshape(B * NW, K, D)
    pos = np.ascontiguousarray(
        np.asarray(pos_emb, np.float32).reshape(W, D)).astype(bf16)
    w1t = np.ascontiguousarray(np.asarray(w1, np.float32).T).astype(bf16)
    w2t = np.ascontiguousarray(np.asarray(w2, np.float32).T).astype(bf16)
    rep = np.zeros((128, W), np.float32)
    rep[:K] = np.kron(np.eye(K, dtype=np.float32), np.ones((1, REP), np.float32))
    rep = rep.astype(bf16)
    idn = np.eye(128, dtype=np.float32).astype(bf16)

    shared = {"pos": pos, "w1t": w1t, "w2t": w2t, "rep": rep, "idn": idn}
    if has_b1:
        shared["b1c"] = np.ascontiguousarray(
            np.asarray(b1, np.float32).reshape(DC, 128).T)
    if has_b2:
        shared["b2r"] = np.asarray(b2, np.float32).reshape(1, D).astype(bf16)
    if has_gb:
        shared["gam"] = np.ascontiguousarray(
            np.asarray(gamma, np.float32).reshape(1, D))
        shared["bet"] = np.ascontiguousarray(
            np.asarray(beta, np.float32).reshape(1, D))

    in_maps = []
    for c in range(N_CORES):
        eng_c = np.ascontiguousarray(
            eng_flat[c * WPC:(c + 1) * WPC].reshape(WPC * K, D)).astype(bf16)
        eng8 = np.zeros((WPC, 128, D), np.float32)
        eng8[:, :K, :] = eng_flat[c * WPC:(c + 1) * WPC]
        eng8 = eng8.reshape(WPC * 128, D).astype(bf16)
        in_maps.append({"eng": eng_c, "eng8": eng8, **shared})
    return in_maps


def kernel(engrams, pos_emb, w1, b1, w2, b2, gamma, beta):
    has_b1 = bool(np.any(np.asarray(b1) != 0))
    has_b2 = bool(np.any(np.asarray(b2) != 0))
    has_gb = bool(np.any(np.asarray(gamma) != 1) or np.any(np.asarray(beta) != 0))

    nc = _get_program(has_b1, has_b2, has_gb)
    in_maps = _make_in_maps(engrams, pos_emb, w1, b1, w2, b2, gamma, beta,
                            has_b1, has_b2, has_gb)
    res = run_bass_kernel_spmd(nc, in_maps, list(range(N_CORES)))
    full = np.concatenate([res.results[c]["out"] for c in range(N_CORES)], axis=0)
    return np.ascontiguousarray(
        full.reshape(B, NW, W, D).astype(np.float32, copy=False))



# revision 8
# speedup vs baseline: 1.3119x; 1.3119x over previous
"""TRN2 Bass/Tile kernel for nn_EngramUpsampler (dense_mlp).

Reference computation:
    x   = repeat_interleave(engrams, 32, axis=2) + pos_emb   # (B, NW, 512, 1024)
    h   = gelu(x @ w1.T + b1, exact)
    x   = x + h @ w2.T + b2
    out = LayerNorm(x) * gamma + beta

Distribution: data-parallel over the B*NW = 64 (batch, window) slices,
8 windows per NeuronCore; weights/pos_emb replicated.

Per-core kernel design:
  * The first matmul factors through the repeat:
        x @ w1.T = repeat(eng @ w1.T) + pos @ w1.T
    eng_proj (128 tokens/core) and pos_projT (512 shared tokens) are computed
    once per core in bf16 -- a ~21x FLOP reduction on matmul 1.
  * pre = repeat(eng_proj) + pos_projT is a broadcast add on DVE in bf16
    (2x throughput); h = gelu(pre) on ScalarE, emitted as fp8 e4m3.
  * Second matmul h @ (64*w2).T runs in fp8 DoubleRow perf mode (K=256 per
    instruction, ~1.8x PE throughput); w2 is pre-scaled by 64 and
    pair-interleaved on the host.  The residual x0 = repeat(eng) + pos joins
    the same PSUM accumulation via 64*R / 64*I matmuls, so PSUM holds 64*x.
  * LayerNorm: bn_stats/bn_aggr on DVE; rsqrt via 3 Newton steps on DVE with
    the 1/64 descale folded into the last step's coefficients; normalization
    on GpSimd reading the fp32 SBUF copy of x.
"""

import numpy as np
import ml_dtypes

import concourse.bass as bass
import concourse.tile as tile
from concourse import bacc, mybir
from concourse.bass_utils import run_bass_kernel_spmd

FP32 = mybir.dt.float32
BF16 = mybir.dt.bfloat16
FP8 = mybir.dt.float8e4
AF = mybir.ActivationFunctionType
OP = mybir.AluOpType
DR = mybir.MatmulPerfMode.DoubleRow

B, NW, K, D, W = 4, 16, 16, 1024, 512
REP = W // K          # 32
N_CORES = 8
WPC = (B * NW) // N_CORES   # 8 windows per core
TOK = WPC * W               # 4096 tokens per core
DC = D // 128               # 8 chunks of the contraction dim
DC2 = DC // 2               # 4 fp8 DoubleRow chunks (K=256 each)
LN_EPS = 1e-5
MM2_SCALE = 64.0            # w2 pre-scale; PSUM holds 64*x
NEWTON_ITERS = 3

_PROGRAM_CACHE: dict = {}


def _build_program(has_b1: bool, has_b2: bool, has_gb: bool):
    """Emit + compile the per-core SPMD program. Same program runs on all 8
    cores; only the DRAM input contents differ."""
    from contextlib import ExitStack

    nc = bacc.Bacc("TRN2", target_bir_lowering=False, debug=False)

    eng_d = nc.dram_tensor("eng", [WPC * K, D], BF16, kind="ExternalInput").ap()
    pos_d = nc.dram_tensor("pos", [W, D], BF16, kind="ExternalInput").ap()
    w1t_d = nc.dram_tensor("w1t", [D, D], BF16, kind="ExternalInput").ap()
    w2p_d = nc.dram_tensor("w2p", [128, DC2 * 2 * D], FP8, kind="ExternalInput").ap()
    rep_d = nc.dram_tensor("rep", [64, 2 * W], BF16, kind="ExternalInput").ap()
    idn_d = nc.dram_tensor("idn", [128, 128], BF16, kind="ExternalInput").ap()
    id64_d = nc.dram_tensor("id64", [128, 128], BF16, kind="ExternalInput").ap()
    if has_b1:
        b1c_d = nc.dram_tensor("b1c", [128, DC], FP32, kind="ExternalInput").ap()
    if has_b2:
        b2r_d = nc.dram_tensor("b2r", [1, D], BF16, kind="ExternalInput").ap()
    if has_gb:
        gam_d = nc.dram_tensor("gam", [1, D], FP32, kind="ExternalInput").ap()
        bet_d = nc.dram_tensor("bet", [1, D], FP32, kind="ExternalInput").ap()
    out_d = nc.dram_tensor("out", [TOK, D], FP32, kind="ExternalOutput").ap()

    with tile.TileContext(nc) as tc, ExitStack() as ctx:
        consts = ctx.enter_context(tc.tile_pool(name="consts", bufs=1))

        def ctile(shape, dtype, tag):
            return consts.tile(shape, dtype, tag=tag, name=tag)

        # ---- persistent SBUF tensors -------------------------------------
        w1t_sb = [ctile([128, D], BF16, f"w1t{i}") for i in range(DC)]
        w2p_sb = [ctile([128, 2 * D], FP8, f"w2p{i}") for i in range(DC2)]
        pos_sb = [ctile([128, D], BF16, f"pos{i}") for i in range(4)]
        eng_all = ctile([128, D], BF16, "eng_all")
        # residual matmul sources: windows 0-3 / 4-7 at partitions 0-63
        # (PE base partition must be 0/32/64, so rows 96-127 are unreachable)
        eng_lo = ctile([64, D], BF16, "eng_lo")
        eng_hi = ctile([64, D], BF16, "eng_hi")
        rep_sb = ctile([64, 2 * W], BF16, "rep")      # 64*R, even/odd windows
        id_sb = ctile([128, 128], BF16, "id_sb")      # plain I (transposes)
        id64_sb = ctile([128, 128], BF16, "id64")     # 64*I (residual pos)
        engT = ctile([128, D], BF16, "engT")          # [d, t] chunks in cols
        posT = [ctile([128, W], BF16, f"posT{i}") for i in range(DC)]
        engproj_all = ctile([128, D], BF16, "engproj")  # [t, e]
        engprojT = ctile([128, D], BF16, "engprojT")    # [e_local, token]
        posproj = [ctile([128, W], BF16, f"ppj{i}") for i in range(DC)]  # [e, w]
        if has_b1:
            b1c_sb = ctile([128, DC], FP32, "b1c")
        if has_b2:
            ones_sb = ctile([1, 128], BF16, "ones")
            b2_sb = ctile([1, D], BF16, "b2")
        if has_gb:
            gam_row = ctile([1, D], FP32, "gamr")
            bet_row = ctile([1, D], FP32, "betr")
            gam_sb = ctile([128, D], FP32, "gam")
            bet_sb = ctile([128, D], FP32, "bet")

        # ---- loads -------------------------------------------------------
        nc.sync.dma_start(id_sb[:], idn_d[:])
        nc.sync.dma_start(eng_all[:], eng_d[:])
        nc.sync.dma_start(eng_lo[:], eng_d[0:64, :])
        nc.sync.dma_start(eng_hi[:], eng_d[64:128, :])
        for i in range(4):
            nc.sync.dma_start(pos_sb[i][:], pos_d[i * 128:(i + 1) * 128, :])
        for i in range(DC):
            nc.sync.dma_start(w1t_sb[i][:], w1t_d[i * 128:(i + 1) * 128, :])
        nc.sync.dma_start(rep_sb[:], rep_d[:])
        nc.sync.dma_start(id64_sb[:], id64_d[:])
        for i in range(DC2):
            nc.sync.dma_start(w2p_sb[i][:], w2p_d[:, i * 2 * D:(i + 1) * 2 * D])
        if has_b1:
            nc.sync.dma_start(b1c_sb[:], b1c_d[:])
        if has_b2:
            nc.gpsimd.memset(ones_sb[:], 1.0)
            nc.sync.dma_start(b2_sb[:], b2r_d[:])
        if has_gb:
            nc.sync.dma_start(gam_row[:], gam_d[:])
            nc.sync.dma_start(bet_row[:], bet_d[:])
            nc.gpsimd.partition_broadcast(gam_sb[:], gam_row[:])
            nc.gpsimd.partition_broadcast(bet_sb[:], bet_row[:])

        # ---- setup: transposes + factored first matmul -------------------
        with tc.tile_pool(name="spsum", bufs=2, space="PSUM") as spsum:
            # engT[d, t]: PE-transpose eng 128x128 blocks
            for dc in range(DC):
                pt = spsum.tile([128, 128], BF16, tag="pt", name="pt")
                nc.tensor.transpose(pt[:], eng_all[:, dc * 128:(dc + 1) * 128], id_sb[:])
                nc.vector.tensor_copy(engT[:, dc * 128:(dc + 1) * 128], pt[:])
            # posT[dc] = [d, w]: transpose pos tiles
            for g in range(4):
                for dc in range(DC):
                    pt = spsum.tile([128, 128], BF16, tag="pt", name="pt")
                    nc.tensor.transpose(
                        pt[:], pos_sb[g][:, dc * 128:(dc + 1) * 128], id_sb[:]
                    )
                    nc.vector.tensor_copy(posT[dc][:, g * 128:(g + 1) * 128], pt[:])
            # eng_proj[t, e] = engT.T @ w1t  (accumulate over d chunks)
            ep = spsum.tile([128, D], FP32, tag="ep", name="ep")
            for half in range(2):
                sl = slice(half * 512, half * 512 + 512)
                for dc in range(DC):
                    nc.tensor.matmul(
                        ep[:, sl],
                        lhsT=engT[:, dc * 128:(dc + 1) * 128],
                        rhs=w1t_sb[dc][:, sl],
                        start=(dc == 0),
                        stop=(dc == DC - 1),
                    )
            nc.vector.tensor_copy(engproj_all[:], ep[:])
            for ec in range(DC):
                pt = spsum.tile([128, 128], BF16, tag="pt", name="pt")
                nc.tensor.transpose(
                    pt[:], engproj_all[:, ec * 128:(ec + 1) * 128], id_sb[:]
                )
                nc.vector.tensor_copy(engprojT[:, ec * 128:(ec + 1) * 128], pt[:])
            # pos_projT[e, w] per e-chunk = w1t_chunk.T @ posT
            for ec in range(DC):
                pp = spsum.tile([128, W], FP32, tag="pp", name="pp")
                for dc in range(DC):
                    nc.tensor.matmul(
                        pp[:],
                        lhsT=w1t_sb[dc][:, ec * 128:(ec + 1) * 128],
                        rhs=posT[dc][:],
                        start=(dc == 0),
                        stop=(dc == DC - 1),
                    )
                nc.vector.tensor_copy(posproj[ec][:], pp[:])

        # ---- main loop ---------------------------------------------------
        pre_pool = ctx.enter_context(tc.tile_pool(name="pre", bufs=3))
        x_pool = ctx.enter_context(tc.tile_pool(name="xps", bufs=3, space="PSUM"))
        h_pool = ctx.enter_context(tc.tile_pool(name="h", bufs=2))
        xs_pool = ctx.enter_context(tc.tile_pool(name="xs", bufs=6))
        st_pool = ctx.enter_context(tc.tile_pool(name="st", bufs=4))
        nt_pool = ctx.enter_context(tc.tile_pool(name="nt", bufs=2))
        out_pool = ctx.enter_context(tc.tile_pool(name="ot", bufs=3))

        for n in range(WPC):
            # -- h = gelu(repeat(eng_proj) + pos_projT) per e-chunk --------
            # h is stored fp8 in DoubleRow pair tiles: [p, j, w] with j the
            # parity of the e-chunk within a K=256 pair.
            h_tiles = [h_pool.tile([128, 2 * W], FP8, tag=f"h{i}", name=f"h{i}")
                       for i in range(DC2)]
            for ec in range(DC):
                pre = pre_pool.tile([128, W], BF16, tag="pre", name="pre")
                # pre[e, w] = engprojT[e, k(w)] + posproj[e, w]; the repeat is
                # a stride-0 inner dim on the engprojT read.
                eng_bc = engprojT[:, ec * 128 + n * K: ec * 128 + (n + 1) * K]
                eng_bc = eng_bc.unsqueeze(2).broadcast_to([128, K, REP])
                nc.vector.tensor_tensor(
                    pre[:].rearrange("p (k r) -> p k r", r=REP),
                    eng_bc,
                    posproj[ec][:].rearrange("p (k r) -> p k r", r=REP),
                    OP.add,
                )
                hv = h_tiles[ec // 2][:, (ec % 2) * W:(ec % 2 + 1) * W]
                bias = b1c_sb[:, ec:ec + 1] if has_b1 else 0.0
                nc.scalar.activation(hv, pre[:], AF.Gelu, bias=bias)

            # -- second matmul (fp8 DoubleRow) + residual, LN stats --------
            mvw = st_pool.tile([128, 2 * 4], FP32, tag="mvw", name="mvw")
            xs_tiles = []
            eng_res = eng_lo if n < 4 else eng_hi
            q, par = (n % 4) // 2, n % 2
            for g in range(4):
                px = x_pool.tile([128, D], FP32, tag="px", name="px")
                for half in range(2):
                    sl = slice(half * 512, half * 512 + 512)
                    for dc2 in range(DC2):
                        h3 = h_tiles[dc2][:].rearrange("p (j w) -> p j w", j=2)
                        w3 = w2p_sb[dc2][:].rearrange("p (j n) -> p j n", j=2)
                        nc.tensor.matmul(
                            px[:, sl],
                            lhsT=h3[:, :, g * 128:(g + 1) * 128],
                            rhs=w3[:, :, sl],
                            start=(dc2 == 0),
                            stop=False,
                            perf_mode=DR,
                        )
                    # residual 64*(repeat(eng) + pos) joins the accumulation
                    nc.tensor.matmul(
                        px[:, sl],
                        lhsT=rep_sb[32 * q:32 * q + 32,
                                    par * W + g * 128:par * W + (g + 1) * 128],
                        rhs=eng_res[32 * q:32 * q + 32, sl],
                        start=False,
                        stop=False,
                    )
                    if has_b2:
                        nc.tensor.matmul(
                            px[:, sl],
                            lhsT=ones_sb[:],
                            rhs=b2_sb[:, sl],
                            start=False,
                            stop=False,
                        )
                    nc.tensor.matmul(
                        px[:, sl],
                        lhsT=id64_sb[:],
                        rhs=pos_sb[g][:, sl],
                        start=False,
                        stop=True,
                    )
                xs = xs_pool.tile([128, D], FP32, tag="xs", name="xs")
                nc.scalar.activation(xs[:], px[:], AF.Identity)
                st = st_pool.tile([128, 12], FP32, tag="st", name="st")
                nc.vector.bn_stats(st[:, 0:6], px[:, 0:512])
                nc.vector.bn_stats(st[:, 6:12], px[:, 512:1024])
                nc.vector.bn_aggr(
                    mvw[:, 2 * g:2 * g + 2],
                    st[:].rearrange("p (n s) -> p n s", s=3),
                )
                xs_tiles.append(xs)

            # -- s = rsqrt(var + eps)/64, Newton on DVE, batched [128,4] ---
            # PSUM holds 64*x, so var_psum = 4096*var; descale by 2^-12
            # before eps, and fold the final 1/64 into the last Newton step.
            vw = nt_pool.tile([128, 4], FP32, tag="vw", name="vw")
            nc.vector.tensor_scalar(vw[:], mvw[:, 1::2], 1.0 / 4096.0, LN_EPS,
                                    OP.mult, OP.add)
            t0 = nt_pool.tile([128, 4], FP32, tag="t0", name="t0")
            nc.vector.tensor_scalar(t0[:], vw[:], 0.5, 0.5, OP.mult, OP.add)
            y = nt_pool.tile([128, 4], FP32, tag="y", name="y")
            nc.vector.reciprocal(y[:], t0[:])
            for it in range(NEWTON_ITERS):
                last = it == NEWTON_ITERS - 1
                f = 1.0 / MM2_SCALE if last else 1.0
                y2 = nt_pool.tile([128, 4], FP32, tag="y2", name="y2")
                nc.vector.tensor_mul(y2[:], y[:], y[:])
                t = nt_pool.tile([128, 4], FP32, tag="t", name="t")
                nc.vector.tensor_mul(t[:], y2[:], vw[:])
                c = nt_pool.tile([128, 4], FP32, tag="c", name="c")
                nc.vector.tensor_scalar(c[:], t[:], -0.5 * f, 1.5 * f,
                                        OP.mult, OP.add)
                yn = nt_pool.tile([128, 4], FP32, tag="y", name="y")
                nc.vector.tensor_mul(yn[:], y[:], c[:])
                y = yn
            nm = nt_pool.tile([128, 4], FP32, tag="nm", name="nm")
            nc.vector.tensor_scalar_mul(nm[:], mvw[:, 0::2], -1.0)

            # -- normalize + store ----------------------------------------
            for g in range(4):
                row0 = n * W + g * 128
                if not has_gb:
                    ot = out_pool.tile([128, D], FP32, tag="ot", name="ot")
                    nc.gpsimd.tensor_scalar(
                        ot[:], xs_tiles[g][:],
                        nm[:, g:g + 1], y[:, g:g + 1], OP.add, OP.mult,
                    )
                else:
                    xn = out_pool.tile([128, D], FP32, tag="xn", name="xn")
                    nc.gpsimd.tensor_scalar(
                        xn[:], xs_tiles[g][:],
                        nm[:, g:g + 1], y[:, g:g + 1], OP.add, OP.mult,
                    )
                    ot = out_pool.tile([128, D], FP32, tag="ot", name="ot")
                    nc.vector.scalar_tensor_tensor(
                        ot[:], xn[:], 1.0, gam_sb[:], OP.mult, OP.mult
                    )
                    nc.vector.tensor_add(ot[:], ot[:], bet_sb[:])
                nc.sync.dma_start(out_d[row0:row0 + 128, :], ot[:])

    nc.compile()
    return nc


def _get_program(has_b1, has_b2, has_gb):
    key = (has_b1, has_b2, has_gb)
    if key not in _PROGRAM_CACHE:
        _PROGRAM_CACHE[key] = _build_program(*key)
    return _PROGRAM_CACHE[key]


def _make_in_maps(engrams, pos_emb, w1, b1, w2, b2, gamma, beta,
                  has_b1, has_b2, has_gb):
    bf16 = ml_dtypes.bfloat16
    e4 = ml_dtypes.float8_e4m3
    eng_flat = np.asarray(engrams, np.float32).reshape(B * NW, K, D)
    pos = np.ascontiguousarray(
        np.asarray(pos_emb, np.float32).reshape(W, D)).astype(bf16)
    w1t = np.ascontiguousarray(np.asarray(w1, np.float32).T).astype(bf16)
    # w2 pre-scaled by 64, pair-interleaved for DoubleRow: [p, dc2, j, n]
    w2t64 = np.clip(np.asarray(w2, np.float32).T * MM2_SCALE, -240, 240)
    w2p = np.ascontiguousarray(
        w2t64.reshape(DC2, 2, 128, D).transpose(2, 0, 1, 3).reshape(128, -1)
    ).astype(e4)
    # rep: [128, 2*W]; partition block 32q holds 64*R for windows 2q/2q+1
    r16 = np.kron(np.eye(K, dtype=np.float32), np.ones((1, REP), np.float32))
    r32 = np.zeros((32, 2 * W), np.float32)
    r32[:K, :W] = MM2_SCALE * r16        # even window
    r32[K:, W:] = MM2_SCALE * r16        # odd window
    rep = np.ascontiguousarray(np.tile(r32, (2, 1))).astype(bf16)
    idn = np.eye(128, dtype=np.float32).astype(bf16)
    id64 = (MM2_SCALE * np.eye(128, dtype=np.float32)).astype(bf16)

    shared = {"pos": pos, "w1t": w1t, "w2p": w2p, "rep": rep, "idn": idn,
              "id64": id64}
    if has_b1:
        shared["b1c"] = np.ascontiguousarray(
            np.asarray(b1, np.float32).reshape(DC, 128).T)
    if has_b2:
        shared["b2r"] = (MM2_SCALE * np.asarray(b2, np.float32)
                         ).reshape(1, D).astype(bf16)
    if has_gb:
        shared["gam"] = np.ascontiguousarray(
            np.asarray(gamma, np.float32).reshape(1, D))
        shared["bet"] = np.ascontiguousarray(
            np.asarray(beta, np.float32).reshape(1, D))

    in_maps = []
    for c in range(N_CORES):
        eng_c = np.ascontiguousarray(
            eng_flat[c * WPC:(c + 1) * WPC].reshape(WPC * K, D)).astype(bf16)
        in_maps.append({"eng": eng_c, **shared})
    return in_maps


def kernel(engrams, pos_emb, w1, b1, w2, b2, gamma, beta):
    has_b1 = bool(np.any(np.asarray(b1) != 0))
    has_b2 = bool(np.any(np.asarray(b2) != 0))
    has_gb = bool(np.any(np.asarray(gamma) != 1) or np.any(np.asarray(beta) != 0))

    nc = _get_program(has_b1, has_b2, has_gb)
    in_maps = _make_in_maps(engrams, pos_emb, w1, b1, w2, b2, gamma, beta,
                            has_b1, has_b2, has_gb)
    res = run_bass_kernel_spmd(nc, in_maps, list(range(N_CORES)))
    full = np.concatenate([res.results[c]["out"] for c in range(N_CORES)], axis=0)
    return np.ascontiguousarray(
        full.reshape(B, NW, W, D).astype(np.float32, copy=False))


# revision 20
# speedup vs baseline: 1.4065x; 1.0721x over previous
"""TRN2 Bass/Tile kernel for nn_EngramUpsampler (dense_mlp).

Reference computation:
    x   = repeat_interleave(engrams, 32, axis=2) + pos_emb   # (B, NW, 512, 1024)
    h   = gelu(x @ w1.T + b1, exact)
    x   = x + h @ w2.T + b2
    out = LayerNorm(x) * gamma + beta

Distribution: data-parallel over the B*NW = 64 (batch, window) slices,
8 windows per NeuronCore; weights/pos_emb replicated.

Per-core kernel design:
  * The first matmul factors through the repeat:
        x @ w1.T = repeat(eng @ w1.T) + pos @ w1.T
    eng_proj (128 tokens/core) and pos_projT (512 shared tokens) are computed
    once per core in bf16 -- a ~21x FLOP reduction on matmul 1.
  * pre = repeat(eng_proj) + pos_projT is a broadcast add on DVE in bf16
    (2x throughput); h = gelu(pre) on ScalarE, emitted as fp8 e4m3.
  * Second matmul h @ (64*w2).T runs in fp8 DoubleRow perf mode (K=256 per
    instruction, ~1.8x PE throughput); w2 is pre-scaled by 64 and
    pair-interleaved on the host.  The residual x0 = repeat(eng) + pos joins
    the same PSUM accumulation via 64*R / 64*I matmuls, so PSUM holds 64*x.
  * LayerNorm: bn_stats/bn_aggr on DVE; rsqrt via 3 Newton steps on DVE with
    the 1/64 descale folded into the last step's coefficients; normalization
    on GpSimd reading the fp32 SBUF copy of x.
"""

import numpy as np
import ml_dtypes

import concourse.bass as bass
import concourse.tile as tile
from concourse import bacc, mybir
from concourse.bass_utils import run_bass_kernel_spmd

FP32 = mybir.dt.float32
BF16 = mybir.dt.bfloat16
FP8 = mybir.dt.float8e4
AF = mybir.ActivationFunctionType
OP = mybir.AluOpType
DR = mybir.MatmulPerfMode.DoubleRow

B, NW, K, D, W = 4, 16, 16, 1024, 512
REP = W // K          # 32
N_CORES = 8
WPC = (B * NW) // N_CORES   # 8 windows per core
TOK = WPC * W               # 4096 tokens per core
DC = D // 128               # 8 chunks of the contraction dim
DC2 = DC // 2               # 4 fp8 DoubleRow chunks (K=256 each)
LN_EPS = 1e-5
MM2_SCALE = 64.0            # w2 pre-scale; PSUM holds 64*x
NEWTON_ITERS = 3

_PROGRAM_CACHE: dict = {}


def _build_program(has_b1: bool, has_b2: bool, has_gb: bool):
    """Emit + compile the per-core SPMD program. Same program runs on all 8
    cores; only the DRAM input contents differ."""
    from contextlib import ExitStack

    nc = bacc.Bacc("TRN2", target_bir_lowering=False, debug=False)

    eng_d = nc.dram_tensor("eng", [WPC * K, D], BF16, kind="ExternalInput").ap()
    eng8_d = nc.dram_tensor("eng8", [WPC * 128, D], BF16, kind="ExternalInput").ap()
    pos_d = nc.dram_tensor("pos", [W, D], BF16, kind="ExternalInput").ap()
    w1t_d = nc.dram_tensor("w1t", [D, D], BF16, kind="ExternalInput").ap()
    w2p_d = nc.dram_tensor("w2p", [128, DC2 * 2 * D], FP8, kind="ExternalInput").ap()
    rep_d = nc.dram_tensor("rep", [128, W], BF16, kind="ExternalInput").ap()
    idn_d = nc.dram_tensor("idn", [128, 128], BF16, kind="ExternalInput").ap()
    id64_d = nc.dram_tensor("id64", [128, 128], BF16, kind="ExternalInput").ap()
    if has_b1:
        b1c_d = nc.dram_tensor("b1c", [128, DC], FP32, kind="ExternalInput").ap()
    if has_b2:
        b2r_d = nc.dram_tensor("b2r", [1, D], BF16, kind="ExternalInput").ap()
    if has_gb:
        gam_d = nc.dram_tensor("gam", [1, D], FP32, kind="ExternalInput").ap()
        bet_d = nc.dram_tensor("bet", [1, D], FP32, kind="ExternalInput").ap()
    out_d = nc.dram_tensor("out", [TOK, D], mybir.dt.float16,
                           kind="ExternalOutput").ap()

    with tile.TileContext(nc) as tc, ExitStack() as ctx:
        consts = ctx.enter_context(tc.tile_pool(name="consts", bufs=1))

        def ctile(shape, dtype, tag):
            return consts.tile(shape, dtype, tag=tag, name=tag)

        # ---- persistent SBUF tensors -------------------------------------
        w1t_sb = [ctile([128, D], BF16, f"w1t{i}") for i in range(DC)]
        w2p_sb = [ctile([128, 2 * D], FP8, f"w2p{i}") for i in range(DC2)]
        pos_sb = [ctile([128, D], BF16, f"pos{i}") for i in range(4)]
        eng_all = ctile([128, D], BF16, "eng_all")
        eng8_sb = [ctile([128, D], BF16, f"engw{i}") for i in range(WPC)]
        rep_sb = ctile([128, W], BF16, "rep")         # 64*R (16 live rows)
        id_sb = ctile([128, 128], BF16, "id_sb")      # plain I (transposes)
        id64_sb = ctile([128, 128], BF16, "id64")     # 64*I (residual pos)
        engT = ctile([128, D], BF16, "engT")          # [d, t] chunks in cols
        posT = [ctile([128, W], BF16, f"posT{i}") for i in range(DC)]
        engproj_all = ctile([128, D], BF16, "engproj")  # [t, e]
        engprojT = ctile([128, D], BF16, "engprojT")    # [e_local, token]
        posproj = [ctile([128, W], BF16, f"ppj{i}") for i in range(DC)]  # [e, w]
        if has_b1:
            b1c_sb = ctile([128, DC], FP32, "b1c")
        if has_b2:
            ones_sb = ctile([1, 128], BF16, "ones")
            b2_sb = ctile([1, D], BF16, "b2")
        if has_gb:
            gam_row = ctile([1, D], FP32, "gamr")
            bet_row = ctile([1, D], FP32, "betr")
            gam_sb = ctile([128, D], FP32, "gam")
            bet_sb = ctile([128, D], FP32, "bet")

        # ---- loads -------------------------------------------------------
        nc.sync.dma_start(id_sb[:], idn_d[:])
        nc.sync.dma_start(eng_all[:], eng_d[:])
        for n in range(WPC):
            nc.sync.dma_start(eng8_sb[n][:], eng8_d[n * 128:(n + 1) * 128, :])
        for i in range(4):
            nc.sync.dma_start(pos_sb[i][:], pos_d[i * 128:(i + 1) * 128, :])
        for i in range(DC):
            nc.sync.dma_start(w1t_sb[i][:], w1t_d[i * 128:(i + 1) * 128, :])
        nc.sync.dma_start(rep_sb[:], rep_d[:])
        nc.sync.dma_start(id64_sb[:], id64_d[:])
        for i in range(DC2):
            nc.sync.dma_start(w2p_sb[i][:], w2p_d[:, i * 2 * D:(i + 1) * 2 * D])
        if has_b1:
            nc.sync.dma_start(b1c_sb[:], b1c_d[:])
        if has_b2:
            nc.gpsimd.memset(ones_sb[:], 1.0)
            nc.sync.dma_start(b2_sb[:], b2r_d[:])
        if has_gb:
            nc.sync.dma_start(gam_row[:], gam_d[:])
            nc.sync.dma_start(bet_row[:], bet_d[:])
            nc.gpsimd.partition_broadcast(gam_sb[:], gam_row[:])
            nc.gpsimd.partition_broadcast(bet_sb[:], bet_row[:])

        # ---- setup: transposes + factored first matmul -------------------
        with tc.tile_pool(name="spsum", bufs=2, space="PSUM") as spsum:
            # engT[d, t]: PE-transpose eng 128x128 blocks
            for dc in range(DC):
                pt = spsum.tile([128, 128], BF16, tag="pt", name="pt")
                nc.tensor.transpose(pt[:], eng_all[:, dc * 128:(dc + 1) * 128], id_sb[:])
                nc.vector.tensor_copy(engT[:, dc * 128:(dc + 1) * 128], pt[:])
            # posT[dc] = [d, w]: transpose pos tiles
            for g in range(4):
                for dc in range(DC):
                    pt = spsum.tile([128, 128], BF16, tag="pt", name="pt")
                    nc.tensor.transpose(
                        pt[:], pos_sb[g][:, dc * 128:(dc + 1) * 128], id_sb[:]
                    )
                    nc.vector.tensor_copy(posT[dc][:, g * 128:(g + 1) * 128], pt[:])
            # eng_proj[t, e] = engT.T @ w1t  (accumulate over d chunks)
            ep = spsum.tile([128, D], FP32, tag="ep", name="ep")
            for half in range(2):
                sl = slice(half * 512, half * 512 + 512)
                for dc in range(DC):
                    nc.tensor.matmul(
                        ep[:, sl],
                        lhsT=engT[:, dc * 128:(dc + 1) * 128],
                        rhs=w1t_sb[dc][:, sl],
                        start=(dc == 0),
                        stop=(dc == DC - 1),
                    )
            nc.vector.tensor_copy(engproj_all[:], ep[:])
            for ec in range(DC):
                pt = spsum.tile([128, 128], BF16, tag="pt", name="pt")
                nc.tensor.transpose(
                    pt[:], engproj_all[:, ec * 128:(ec + 1) * 128], id_sb[:]
                )
                nc.vector.tensor_copy(engprojT[:, ec * 128:(ec + 1) * 128], pt[:])
            # pos_projT[e, w] per e-chunk = w1t_chunk.T @ posT
            for ec in range(DC):
                pp = spsum.tile([128, W], FP32, tag="pp", name="pp")
                for dc in range(DC):
                    nc.tensor.matmul(
                        pp[:],
                        lhsT=w1t_sb[dc][:, ec * 128:(ec + 1) * 128],
                        rhs=posT[dc][:],
                        start=(dc == 0),
                        stop=(dc == DC - 1),
                    )
                nc.vector.tensor_copy(posproj[ec][:], pp[:])

        # ---- main loop ---------------------------------------------------
        pre_pool = ctx.enter_context(tc.tile_pool(name="pre", bufs=4))
        x_pool = ctx.enter_context(tc.tile_pool(name="xps", bufs=3, space="PSUM"))
        h_pool = ctx.enter_context(tc.tile_pool(name="h", bufs=2))
        xs_pool = ctx.enter_context(tc.tile_pool(name="xs", bufs=8))
        st_pool = ctx.enter_context(tc.tile_pool(name="st", bufs=4))
        nt_pool = ctx.enter_context(tc.tile_pool(name="nt", bufs=2))
        out_pool = ctx.enter_context(tc.tile_pool(name="ot", bufs=4))

        for n in range(WPC):
            # -- h = gelu(repeat(eng_proj) + pos_projT) per e-chunk --------
            # h is stored fp8 in DoubleRow pair tiles: [p, j, w] with j the
            # parity of the e-chunk within a K=256 pair.
            h_tiles = [h_pool.tile([128, 2 * W], FP8, tag=f"h{i}", name=f"h{i}")
                       for i in range(DC2)]
            for ec in range(DC):
                pre = pre_pool.tile([128, W], BF16, tag="pre", name="pre")
                # pre[e, w] = engprojT[e, k(w)] + posproj[e, w]; the repeat is
                # a stride-0 inner dim on the engprojT read.
                eng_bc = engprojT[:, ec * 128 + n * K: ec * 128 + (n + 1) * K]
                eng_bc = eng_bc.unsqueeze(2).broadcast_to([128, K, REP])
                nc.vector.tensor_tensor(
                    pre[:].rearrange("p (k r) -> p k r", r=REP),
                    eng_bc,
                    posproj[ec][:].rearrange("p (k r) -> p k r", r=REP),
                    OP.add,
                )
                hv = h_tiles[ec // 2][:, (ec % 2) * W:(ec % 2 + 1) * W]
                bias = b1c_sb[:, ec:ec + 1] if has_b1 else 0.0
                nc.scalar.activation(hv, pre[:], AF.Gelu, bias=bias)

            # -- second matmul (fp8 DoubleRow) + residual, LN stats --------
            mvw = st_pool.tile([128, 2 * 4], FP32, tag="mvw", name="mvw")
            xs_tiles = []
            for g in range(4):
                px = x_pool.tile([128, D], FP32, tag="px", name="px")
                for half in range(2):
                    sl = slice(half * 512, half * 512 + 512)
                    for dc2 in range(DC2):
                        h3 = h_tiles[dc2][:].rearrange("p (j w) -> p j w", j=2)
                        w3 = w2p_sb[dc2][:].rearrange("p (j n) -> p j n", j=2)
                        nc.tensor.matmul(
                            px[:, sl],
                            lhsT=h3[:, :, g * 128:(g + 1) * 128],
                            rhs=w3[:, :, sl],
                            start=(dc2 == 0),
                            stop=False,
                            perf_mode=DR,
                        )
                    # residual 64*(repeat(eng) + pos) joins the accumulation
                    nc.tensor.matmul(
                        px[:, sl],
                        lhsT=rep_sb[:, g * 128:(g + 1) * 128],
                        rhs=eng8_sb[n][:, sl],
                        start=False,
                        stop=False,
                    )
                    if has_b2:
                        nc.tensor.matmul(
                            px[:, sl],
                            lhsT=ones_sb[:],
                            rhs=b2_sb[:, sl],
                            start=False,
                            stop=False,
                        )
                    nc.tensor.matmul(
                        px[:, sl],
                        lhsT=id64_sb[:],
                        rhs=pos_sb[g][:, sl],
                        start=False,
                        stop=True,
                    )
                xs = xs_pool.tile([128, D], FP32, tag="xs", name="xs")
                nc.scalar.activation(xs[:], px[:], AF.Identity)
                st = st_pool.tile([128, 12], FP32, tag="st", name="st")
                nc.vector.bn_stats(st[:, 0:6], px[:, 0:512])
                nc.vector.bn_stats(st[:, 6:12], px[:, 512:1024])
                nc.vector.bn_aggr(
                    mvw[:, 2 * g:2 * g + 2],
                    st[:].rearrange("p (n s) -> p n s", s=3),
                )
                xs_tiles.append(xs)

            # -- s = rsqrt(var + eps)/64, Newton on DVE, batched [128,4] ---
            # PSUM holds 64*x, so var_psum = 4096*var; descale by 2^-12
            # before eps, and fold the final 1/64 into the last Newton step.
            vw = nt_pool.tile([128, 4], FP32, tag="vw", name="vw")
            nc.vector.tensor_scalar(vw[:], mvw[:, 1::2], 1.0 / 4096.0, LN_EPS,
                                    OP.mult, OP.add)
            t0 = nt_pool.tile([128, 4], FP32, tag="t0", name="t0")
            nc.vector.tensor_scalar(t0[:], vw[:], 0.5, 0.5, OP.mult, OP.add)
            y = nt_pool.tile([128, 4], FP32, tag="y", name="y")
            nc.vector.reciprocal(y[:], t0[:])
            for it in range(NEWTON_ITERS):
                last = it == NEWTON_ITERS - 1
                f = 1.0 / MM2_SCALE if last else 1.0
                y2 = nt_pool.tile([128, 4], FP32, tag="y2", name="y2")
                nc.vector.tensor_mul(y2[:], y[:], y[:])
                t = nt_pool.tile([128, 4], FP32, tag="t", name="t")
                nc.vector.tensor_mul(t[:], y2[:], vw[:])
                c = nt_pool.tile([128, 4], FP32, tag="c", name="c")
                nc.vector.tensor_scalar(c[:], t[:], -0.5 * f, 1.5 * f,
                                        OP.mult, OP.add)
                yn = nt_pool.tile([128, 4], FP32, tag="y", name="y")
                nc.vector.tensor_mul(yn[:], y[:], c[:])
                y = yn
            nm = nt_pool.tile([128, 4], FP32, tag="nm", name="nm")
            nc.vector.tensor_scalar_mul(nm[:], mvw[:, 0::2], -1.0)

            # -- normalize + store ----------------------------------------
            for g in range(4):
                row0 = n * W + g * 128
                if not has_gb:
                    ot = out_pool.tile([128, D], mybir.dt.float16,
                                       tag="ot", name="ot")
                    nc.gpsimd.tensor_scalar(
                        ot[:], xs_tiles[g][:],
                        nm[:, g:g + 1], y[:, g:g + 1], OP.add, OP.mult,
                    )
                else:
                    xn = out_pool.tile([128, D], FP32, tag="xn", name="xn")
                    nc.gpsimd.tensor_scalar(
                        xn[:], xs_tiles[g][:],
                        nm[:, g:g + 1], y[:, g:g + 1], OP.add, OP.mult,
                    )
                    ot = out_pool.tile([128, D], mybir.dt.float16,
                                       tag="ot", name="ot")
                    nc.vector.scalar_tensor_tensor(
                        ot[:], xn[:], 1.0, gam_sb[:], OP.mult, OP.mult
                    )
                    nc.vector.tensor_add(ot[:], ot[:], bet_sb[:])
                nc.sync.dma_start(out_d[row0:row0 + 128, :], ot[:])

    nc.compile()
    return nc


def _get_program(has_b1, has_b2, has_gb):
    key = (has_b1, has_b2, has_gb)
    if key not in _PROGRAM_CACHE:
        _PROGRAM_CACHE[key] = _build_program(*key)
    return _PROGRAM_CACHE[key]


def _make_in_maps(engrams, pos_emb, w1, b1, w2, b2, gamma, beta,
                  has_b1, has_b2, has_gb):
    bf16 = ml_dtypes.bfloat16
    e4 = ml_dtypes.float8_e4m3
    eng_flat = np.asarray(engrams, np.float32).reshape(B * NW, K, D)
    pos = np.ascontiguousarray(
        np.asarray(pos_emb, np.float32).reshape(W, D)).astype(bf16)
    w1t = np.ascontiguousarray(np.asarray(w1, np.float32).T).astype(bf16)
    # w2 pre-scaled by 64, pair-interleaved for DoubleRow: [p, dc2, j, n]
    w2t64 = np.clip(np.asarray(w2, np.float32).T * MM2_SCALE, -240, 240)
    w2p = np.ascontiguousarray(
        w2t64.reshape(DC2, 2, 128, D).transpose(2, 0, 1, 3).reshape(128, -1)
    ).astype(e4)
    # rep: [128, W] = 64*R, 16 live rows
    r16 = np.kron(np.eye(K, dtype=np.float32), np.ones((1, REP), np.float32))
    rep = np.zeros((128, W), np.float32)
    rep[:K] = MM2_SCALE * r16
    rep = rep.astype(bf16)
    idn = np.eye(128, dtype=np.float32).astype(bf16)
    id64 = (MM2_SCALE * np.eye(128, dtype=np.float32)).astype(bf16)

    shared = {"pos": pos, "w1t": w1t, "w2p": w2p, "rep": rep, "idn": idn,
              "id64": id64}
    if has_b1:
        shared["b1c"] = np.ascontiguousarray(
            np.asarray(b1, np.float32).reshape(DC, 128).T)
    if has_b2:
        shared["b2r"] = (MM2_SCALE * np.asarray(b2, np.float32)
                         ).reshape(1, D).astype(bf16)
    if has_gb:
        shared["gam"] = np.ascontiguousarray(
            np.asarray(gamma, np.float32).reshape(1, D))
        shared["bet"] = np.ascontiguousarray(
            np.asarray(beta, np.float32).reshape(1, D))

    in_maps = []
    for c in range(N_CORES):
        eng_c = np.ascontiguousarray(
            eng_flat[c * WPC:(c + 1) * WPC].reshape(WPC * K, D)).astype(bf16)
        eng8 = np.zeros((WPC, 128, D), np.float32)
        eng8[:, :K, :] = eng_flat[c * WPC:(c + 1) * WPC]
        eng8 = eng8.reshape(WPC * 128, D).astype(bf16)
        in_maps.append({"eng": eng_c, "eng8": eng8, **shared})
    return in_maps


def kernel(engrams, pos_emb, w1, b1, w2, b2, gamma, beta):
    has_b1 = bool(np.any(np.asarray(b1) != 0))
    has_b2 = bool(np.any(np.asarray(b2) != 0))
    has_gb = bool(np.any(np.asarray(gamma) != 1) or np.any(np.asarray(beta) != 0))

    nc = _get_program(has_b1, has_b2, has_gb)
    in_maps = _make_in_maps(engrams, pos_emb, w1, b1, w2, b2, gamma, beta,
                            has_b1, has_b2, has_gb)
    res = run_bass_kernel_spmd(nc, in_maps, list(range(N_CORES)))
    full = np.concatenate([res.results[c]["out"] for c in range(N_CORES)], axis=0)
    return np.ascontiguousarray(
        full.reshape(B, NW, W, D).astype(np.float32, copy=False))


# revision 34
# speedup vs baseline: 1.4357x; 1.0208x over previous
"""TRN2 Bass/Tile kernel for nn_EngramUpsampler (dense_mlp).

Reference computation:
    x   = repeat_interleave(engrams, 32, axis=2) + pos_emb   # (B, NW, 512, 1024)
    h   = gelu(x @ w1.T + b1, exact)
    x   = x + h @ w2.T + b2
    out = LayerNorm(x) * gamma + beta

Distribution: data-parallel over the B*NW = 64 (batch, window) slices,
8 windows per NeuronCore; weights/pos_emb replicated.

Per-core kernel design:
  * The first matmul factors through the repeat:
        x @ w1.T = repeat(eng @ w1.T) + pos @ w1.T
    eng_proj (128 tokens/core) and pos_projT (512 shared tokens) are computed
    once per core in bf16 -- a ~21x FLOP reduction on matmul 1.
  * pre = repeat(eng_proj) + pos_projT is a broadcast add on DVE in bf16
    (2x throughput); h = gelu(pre) on ScalarE, emitted as fp8 e4m3.
  * Second matmul h @ (64*w2).T runs in fp8 DoubleRow perf mode (K=256 per
    instruction, ~1.8x PE throughput); w2 is pre-scaled by 64 and
    pair-interleaved on the host.  The residual x0 = repeat(eng) + pos joins
    the same PSUM accumulation via 64*R / 64*I matmuls, so PSUM holds 64*x.
  * LayerNorm: bn_stats/bn_aggr on DVE; rsqrt via 3 Newton steps on DVE with
    the 1/64 descale folded into the last step's coefficients; normalization
    on GpSimd reading the fp32 SBUF copy of x.
"""

import numpy as np
import ml_dtypes

import concourse.bass as bass
import concourse.tile as tile
from concourse import bacc, mybir
from concourse.bass_utils import run_bass_kernel_spmd

FP32 = mybir.dt.float32
BF16 = mybir.dt.bfloat16
FP8 = mybir.dt.float8e4
AF = mybir.ActivationFunctionType
OP = mybir.AluOpType
DR = mybir.MatmulPerfMode.DoubleRow

B, NW, K, D, W = 4, 16, 16, 1024, 512
REP = W // K          # 32
N_CORES = 8
WPC = (B * NW) // N_CORES   # 8 windows per core
TOK = WPC * W               # 4096 tokens per core
DC = D // 128               # 8 chunks of the contraction dim
DC2 = DC // 2               # 4 fp8 DoubleRow chunks (K=256 each)
LN_EPS = 1e-5
MM2_SCALE = 64.0            # w2 pre-scale; PSUM holds 64*x
NEWTON_ITERS = 3

_PROGRAM_CACHE: dict = {}


def _build_program(has_b1: bool, has_b2: bool, has_gb: bool):
    """Emit + compile the per-core SPMD program. Same program runs on all 8
    cores; only the DRAM input contents differ."""
    from contextlib import ExitStack

    nc = bacc.Bacc("TRN2", target_bir_lowering=False, debug=False)

    eng_d = nc.dram_tensor("eng", [WPC * K, D], BF16, kind="ExternalInput").ap()
    x064_d = nc.dram_tensor("x064", [TOK, D], BF16, kind="ExternalInput").ap()
    pos_d = nc.dram_tensor("pos", [W, D], BF16, kind="ExternalInput").ap()
    w1t_d = nc.dram_tensor("w1t", [D, D], BF16, kind="ExternalInput").ap()
    w2p_d = nc.dram_tensor("w2p", [128, DC2 * 2 * D], FP8, kind="ExternalInput").ap()
    idn_d = nc.dram_tensor("idn", [128, 128], BF16, kind="ExternalInput").ap()
    if has_b1:
        b1c_d = nc.dram_tensor("b1c", [128, DC], FP32, kind="ExternalInput").ap()
    if has_b2:
        b2r_d = nc.dram_tensor("b2r", [1, D], BF16, kind="ExternalInput").ap()
    if has_gb:
        gam_d = nc.dram_tensor("gam", [1, D], FP32, kind="ExternalInput").ap()
        bet_d = nc.dram_tensor("bet", [1, D], FP32, kind="ExternalInput").ap()
    out_d = nc.dram_tensor("out", [TOK, D], mybir.dt.float16,
                           kind="ExternalOutput").ap()

    with tile.TileContext(nc) as tc, ExitStack() as ctx:
        consts = ctx.enter_context(tc.tile_pool(name="consts", bufs=1))

        def ctile(shape, dtype, tag):
            return consts.tile(shape, dtype, tag=tag, name=tag)

        # ---- persistent SBUF tensors -------------------------------------
        w1t_sb = [ctile([128, D], BF16, f"w1t{i}") for i in range(DC)]
        w2p_sb = [ctile([128, 2 * D], FP8, f"w2p{i}") for i in range(DC2)]
        pos_sb = [ctile([128, D], BF16, f"pos{i}") for i in range(4)]
        eng_all = ctile([128, D], BF16, "eng_all")
        id_sb = ctile([128, 128], BF16, "id_sb")      # plain I (transposes)
        engT = ctile([128, D], BF16, "engT")          # [d, t] chunks in cols
        posT = [ctile([128, W], BF16, f"posT{i}") for i in range(DC)]
        engproj_all = ctile([128, D], BF16, "engproj")  # [t, e]
        engprojT = ctile([128, D], BF16, "engprojT")    # [e_local, token]
        posproj = [ctile([128, W], BF16, f"ppj{i}") for i in range(DC)]  # [e, w]
        if has_b1:
            b1c_sb = ctile([128, DC], FP32, "b1c")
        if has_b2:
            ones_sb = ctile([1, 128], BF16, "ones")
            b2_sb = ctile([1, D], BF16, "b2")
        if has_gb:
            gam_row = ctile([1, D], FP32, "gamr")
            bet_row = ctile([1, D], FP32, "betr")
            gam_sb = ctile([128, D], FP32, "gam")
            bet_sb = ctile([128, D], FP32, "bet")

        # ---- loads -------------------------------------------------------
        nc.sync.dma_start(id_sb[:], idn_d[:])
        nc.sync.dma_start(eng_all[:], eng_d[:])
        for i in range(4):
            nc.sync.dma_start(pos_sb[i][:], pos_d[i * 128:(i + 1) * 128, :])
        for i in range(DC):
            nc.sync.dma_start(w1t_sb[i][:], w1t_d[i * 128:(i + 1) * 128, :])
        for i in range(DC2):
            nc.sync.dma_start(w2p_sb[i][:], w2p_d[:, i * 2 * D:(i + 1) * 2 * D])
        if has_b1:
            nc.sync.dma_start(b1c_sb[:], b1c_d[:])
        if has_b2:
            nc.gpsimd.memset(ones_sb[:], 1.0)
            nc.sync.dma_start(b2_sb[:], b2r_d[:])
        if has_gb:
            nc.sync.dma_start(gam_row[:], gam_d[:])
            nc.sync.dma_start(bet_row[:], bet_d[:])
            nc.gpsimd.partition_broadcast(gam_sb[:], gam_row[:])
            nc.gpsimd.partition_broadcast(bet_sb[:], bet_row[:])

        # ---- setup: transposes + factored first matmul -------------------
        with tc.tile_pool(name="spsum", bufs=2, space="PSUM") as spsum:
            # engT[d, t]: PE-transpose eng 128x128 blocks
            for dc in range(DC):
                pt = spsum.tile([128, 128], BF16, tag="pt", name="pt")
                nc.tensor.transpose(pt[:], eng_all[:, dc * 128:(dc + 1) * 128], id_sb[:])
                nc.vector.tensor_copy(engT[:, dc * 128:(dc + 1) * 128], pt[:])
            # posT[dc] = [d, w]: transpose pos tiles
            for g in range(4):
                for dc in range(DC):
                    pt = spsum.tile([128, 128], BF16, tag="pt", name="pt")
                    nc.tensor.transpose(
                        pt[:], pos_sb[g][:, dc * 128:(dc + 1) * 128], id_sb[:]
                    )
                    nc.vector.tensor_copy(posT[dc][:, g * 128:(g + 1) * 128], pt[:])
            # eng_proj[t, e] = engT.T @ w1t  (accumulate over d chunks)
            ep = spsum.tile([128, D], FP32, tag="ep", name="ep")
            for half in range(2):
                sl = slice(half * 512, half * 512 + 512)
                for dc in range(DC):
                    nc.tensor.matmul(
                        ep[:, sl],
                        lhsT=engT[:, dc * 128:(dc + 1) * 128],
                        rhs=w1t_sb[dc][:, sl],
                        start=(dc == 0),
                        stop=(dc == DC - 1),
                    )
            nc.vector.tensor_copy(engproj_all[:], ep[:])
            for ec in range(DC):
                pt = spsum.tile([128, 128], BF16, tag="pt", name="pt")
                nc.tensor.transpose(
                    pt[:], engproj_all[:, ec * 128:(ec + 1) * 128], id_sb[:]
                )
                nc.vector.tensor_copy(engprojT[:, ec * 128:(ec + 1) * 128], pt[:])
            # pos_projT[e, w] per e-chunk = w1t_chunk.T @ posT
            for ec in range(DC):
                pp = spsum.tile([128, W], FP32, tag="pp", name="pp")
                for dc in range(DC):
                    nc.tensor.matmul(
                        pp[:],
                        lhsT=w1t_sb[dc][:, ec * 128:(ec + 1) * 128],
                        rhs=posT[dc][:],
                        start=(dc == 0),
                        stop=(dc == DC - 1),
                    )
                nc.vector.tensor_copy(posproj[ec][:], pp[:])

        # ---- main loop ---------------------------------------------------
        pre_pool = ctx.enter_context(tc.tile_pool(name="pre", bufs=4))
        x_pool = ctx.enter_context(tc.tile_pool(name="xps", bufs=3, space="PSUM"))
        h_pool = ctx.enter_context(tc.tile_pool(name="h", bufs=2))
        x0_pool = ctx.enter_context(tc.tile_pool(name="x0", bufs=6))
        xs_pool = ctx.enter_context(tc.tile_pool(name="xs", bufs=8))
        st_pool = ctx.enter_context(tc.tile_pool(name="st", bufs=4))
        nt_pool = ctx.enter_context(tc.tile_pool(name="nt", bufs=2))
        out_pool = ctx.enter_context(tc.tile_pool(name="ot", bufs=4))

        for n in range(WPC):
            # -- h = gelu(repeat(eng_proj) + pos_projT) per e-chunk --------
            # h is stored fp8 in DoubleRow pair tiles: [p, j, w] with j the
            # parity of the e-chunk within a K=256 pair.
            h_tiles = [h_pool.tile([128, 2 * W], FP8, tag=f"h{i}", name=f"h{i}")
                       for i in range(DC2)]
            for ec in range(DC):
                pre = pre_pool.tile([128, W], BF16, tag="pre", name="pre")
                # pre[e, w] = engprojT[e, k(w)] + posproj[e, w]; the repeat is
                # a stride-0 inner dim on the engprojT read.
                eng_bc = engprojT[:, ec * 128 + n * K: ec * 128 + (n + 1) * K]
                eng_bc = eng_bc.unsqueeze(2).broadcast_to([128, K, REP])
                nc.gpsimd.tensor_tensor(
                    pre[:].rearrange("p (k r) -> p k r", r=REP),
                    eng_bc,
                    posproj[ec][:].rearrange("p (k r) -> p k r", r=REP),
                    OP.add,
                )
                hv = h_tiles[ec // 2][:, (ec % 2) * W:(ec % 2 + 1) * W]
                bias = b1c_sb[:, ec:ec + 1] if has_b1 else 0.0
                nc.scalar.activation(hv, pre[:], AF.Gelu, bias=bias)

            # -- second matmul (fp8 DoubleRow) + residual, LN stats --------
            mvw = st_pool.tile([128, 2 * 4], FP32, tag="mvw", name="mvw")
            xs_tiles = []
            for g in range(4):
                x0 = x0_pool.tile([128, D], BF16, tag="x0", name="x0")
                row0 = n * W + g * 128
                nc.sync.dma_start(x0[:], x064_d[row0:row0 + 128, :])
                px = x_pool.tile([128, D], FP32, tag="px", name="px")
                for half in range(2):
                    sl = slice(half * 512, half * 512 + 512)
                    for dc2 in range(DC2):
                        h3 = h_tiles[dc2][:].rearrange("p (j w) -> p j w", j=2)
                        w3 = w2p_sb[dc2][:].rearrange("p (j n) -> p j n", j=2)
                        nc.tensor.matmul(
                            px[:, sl],
                            lhsT=h3[:, :, g * 128:(g + 1) * 128],
                            rhs=w3[:, :, sl],
                            start=(dc2 == 0),
                            stop=(dc2 == DC2 - 1) and not has_b2,
                            perf_mode=DR,
                        )
                    if has_b2:
                        nc.tensor.matmul(
                            px[:, sl],
                            lhsT=ones_sb[:],
                            rhs=b2_sb[:, sl],
                            start=False,
                            stop=True,
                        )
                # xs = 64*x = mm2 psum + 64*(repeat(eng) + pos), in bf16
                xs = xs_pool.tile([128, D], BF16, tag="xs", name="xs")
                nc.vector.tensor_tensor(xs[:], px[:], x0[:], OP.add)
                st = st_pool.tile([128, 12], FP32, tag="st", name="st")
                nc.vector.bn_stats(st[:, 0:6], xs[:, 0:512])
                nc.vector.bn_stats(st[:, 6:12], xs[:, 512:1024])
                nc.vector.bn_aggr(
                    mvw[:, 2 * g:2 * g + 2],
                    st[:].rearrange("p (n s) -> p n s", s=3),
                )
                xs_tiles.append(xs)

            # -- s = rsqrt(var + eps)/64, Newton on DVE, batched [128,4] ---
            # PSUM holds 64*x, so var_psum = 4096*var; descale by 2^-12
            # before eps, and fold the final 1/64 into the last Newton step.
            vw = nt_pool.tile([128, 4], FP32, tag="vw", name="vw")
            nc.vector.tensor_scalar(vw[:], mvw[:, 1::2], 1.0 / 4096.0, LN_EPS,
                                    OP.mult, OP.add)
            t0 = nt_pool.tile([128, 4], FP32, tag="t0", name="t0")
            nc.vector.tensor_scalar(t0[:], vw[:], 0.5, 0.5, OP.mult, OP.add)
            y = nt_pool.tile([128, 4], FP32, tag="y", name="y")
            nc.vector.reciprocal(y[:], t0[:])
            for it in range(NEWTON_ITERS):
                last = it == NEWTON_ITERS - 1
                f = 1.0 / MM2_SCALE if last else 1.0
                y2 = nt_pool.tile([128, 4], FP32, tag="y2", name="y2")
                nc.vector.tensor_mul(y2[:], y[:], y[:])
                t = nt_pool.tile([128, 4], FP32, tag="t", name="t")
                nc.vector.tensor_mul(t[:], y2[:], vw[:])
                c = nt_pool.tile([128, 4], FP32, tag="c", name="c")
                nc.vector.tensor_scalar(c[:], t[:], -0.5 * f, 1.5 * f,
                                        OP.mult, OP.add)
                yn = nt_pool.tile([128, 4], FP32, tag="y", name="y")
                nc.vector.tensor_mul(yn[:], y[:], c[:])
                y = yn
            # bias = -mean * s, so out = xs*s + bias = (x - mu)*rsqrt(var+eps)
            bb = nt_pool.tile([128, 4], FP32, tag="bb", name="bb")
            nc.vector.tensor_mul(bb[:], mvw[:, 0::2], y[:])
            nm = nt_pool.tile([128, 4], FP32, tag="nm", name="nm")
            nc.vector.tensor_scalar_mul(nm[:], bb[:], -1.0)

            # -- normalize (ScalarE: out = xs*scale + bias) + store --------
            for g in range(4):
                row0 = n * W + g * 128
                if not has_gb:
                    ot = out_pool.tile([128, D], mybir.dt.float16,
                                       tag="ot", name="ot")
                    nc.scalar.activation(
                        ot[:], xs_tiles[g][:], AF.Identity,
                        bias=nm[:, g:g + 1], scale=y[:, g:g + 1],
                    )
                else:
                    xn = out_pool.tile([128, D], FP32, tag="xn", name="xn")
                    nc.scalar.activation(
                        xn[:], xs_tiles[g][:], AF.Identity,
                        bias=nm[:, g:g + 1], scale=y[:, g:g + 1],
                    )
                    ot = out_pool.tile([128, D], mybir.dt.float16,
                                       tag="ot", name="ot")
                    nc.vector.scalar_tensor_tensor(
                        ot[:], xn[:], 1.0, gam_sb[:], OP.mult, OP.mult
                    )
                    nc.vector.tensor_add(ot[:], ot[:], bet_sb[:])
                nc.sync.dma_start(out_d[row0:row0 + 128, :], ot[:])

    nc.compile()
    return nc


def _get_program(has_b1, has_b2, has_gb):
    key = (has_b1, has_b2, has_gb)
    if key not in _PROGRAM_CACHE:
        _PROGRAM_CACHE[key] = _build_program(*key)
    return _PROGRAM_CACHE[key]


def _make_in_maps(engrams, pos_emb, w1, b1, w2, b2, gamma, beta,
                  has_b1, has_b2, has_gb):
    bf16 = ml_dtypes.bfloat16
    e4 = ml_dtypes.float8_e4m3
    eng_flat = np.asarray(engrams, np.float32).reshape(B * NW, K, D)
    pos = np.ascontiguousarray(
        np.asarray(pos_emb, np.float32).reshape(W, D)).astype(bf16)
    w1t = np.ascontiguousarray(np.asarray(w1, np.float32).T).astype(bf16)
    # w2 pre-scaled by 64, pair-interleaved for DoubleRow: [p, dc2, j, n]
    w2t64 = np.clip(np.asarray(w2, np.float32).T * MM2_SCALE, -240, 240)
    w2p = np.ascontiguousarray(
        w2t64.reshape(DC2, 2, 128, D).transpose(2, 0, 1, 3).reshape(128, -1)
    ).astype(e4)
    idn = np.eye(128, dtype=np.float32).astype(bf16)

    shared = {"pos": pos, "w1t": w1t, "w2p": w2p, "idn": idn}
    if has_b1:
        shared["b1c"] = np.ascontiguousarray(
            np.asarray(b1, np.float32).reshape(DC, 128).T)
    if has_b2:
        shared["b2r"] = (MM2_SCALE * np.asarray(b2, np.float32)
                         ).reshape(1, D).astype(bf16)
    if has_gb:
        shared["gam"] = np.ascontiguousarray(
            np.asarray(gamma, np.float32).reshape(1, D))
        shared["bet"] = np.ascontiguousarray(
            np.asarray(beta, np.float32).reshape(1, D))

    # residual tiles 64*(repeat(eng) + pos), streamed to SBUF during the loop
    posf = np.asarray(pos_emb, np.float32).reshape(1, W, D)
    in_maps = []
    for c in range(N_CORES):
        eng_c = np.ascontiguousarray(
            eng_flat[c * WPC:(c + 1) * WPC].reshape(WPC * K, D)).astype(bf16)
        x064 = MM2_SCALE * (
            np.repeat(eng_flat[c * WPC:(c + 1) * WPC], REP, axis=1) + posf)
        x064 = np.ascontiguousarray(x064.reshape(TOK, D)).astype(bf16)
        in_maps.append({"eng": eng_c, "x064": x064, **shared})
    return in_maps


def kernel(engrams, pos_emb, w1, b1, w2, b2, gamma, beta):
    has_b1 = bool(np.any(np.asarray(b1) != 0))
    has_b2 = bool(np.any(np.asarray(b2) != 0))
    has_gb = bool(np.any(np.asarray(gamma) != 1) or np.any(np.asarray(beta) != 0))

    nc = _get_program(has_b1, has_b2, has_gb)
    in_maps = _make_in_maps(engrams, pos_emb, w1, b1, w2, b2, gamma, beta,
                            has_b1, has_b2, has_gb)
    res = run_bass_kernel_spmd(nc, in_maps, list(range(N_CORES)))
    full = np.concatenate([res.results[c]["out"] for c in range(N_CORES)], axis=0)
    return np.ascontiguousarray(
        full.reshape(B, NW, W, D).astype(np.float32, copy=False))
